# revision 19
# baseline (speedup 1.0000x reference)
"""Trainium2 Bass kernel for nn_Brick_Wall (brick-wall gate-layer gradient).

Math: for each gate g the 4x4 antisymmetric E(chi) splits over so(4) =
su(2)+su(2) as E = L(a) + R(b) (left/right quaternion multiplications), so
expm(E) = L(exp_H a) R(exp_H b) in closed form, and the Frechet derivative
contracts against the per-gate matrix Z = (W C^T - C^T W) U down to two
per-gate 4-vectors kappa/lambda.

This version runs the ENTIRE per-gate computation on the Vector engine (DVE)
only: sinc/cos/(cos-sinc)/h^2 are evaluated as degree-8 polynomials in h^2
via a single tensor_tensor_scan (Horner recurrence along the free dim), so
no Scalar-engine activations, no activation-table loads, and no cross-engine
compute synchronization are needed.

Sharding: gates (2048) split contiguously across 8 cores (256 = 2 blocks of
128 partitions each). Host does layout marshaling only (diag-block extraction,
signed column permutations, reshapes); all arithmetic runs on-device.
"""
import sys

for _p in ("/opt/trn_rl_repo",):
    if _p not in sys.path:
        sys.path.insert(0, _p)

import numpy as np

import concourse.bacc as bacc
import concourse.bass as bass
import concourse.tile as tile
from concourse import mybir
from concourse.bass_utils import run_bass_kernel_spmd

F32 = np.float32
P = 128          # partitions (gates per block)
B = 2            # gate blocks per core
NCORES = 8
GPC = P * B      # gates per core
DT = mybir.dt.float32

# ---------------- constant tables (quaternion algebra) ----------------
_Q = np.zeros((4, 4, 4))
for (a, b), (c, s) in {
    (0, 0): (0, 1), (0, 1): (1, 1), (0, 2): (2, 1), (0, 3): (3, 1),
    (1, 0): (1, 1), (1, 1): (0, -1), (1, 2): (3, 1), (1, 3): (2, -1),
    (2, 0): (2, 1), (2, 1): (3, -1), (2, 2): (0, -1), (2, 3): (1, 1),
    (3, 0): (3, 1), (3, 1): (2, 1), (3, 2): (1, -1), (3, 3): (0, -1),
}.items():
    _Q[a, b, c] = s

G_SGN = np.zeros((4, 4))   # R(qbar)[k,j] = G_SGN[k,j] * q_{k xor j}
H_SGN = np.zeros((4, 4))   # L(pbar)[i,k] = H_SGN[k,i] * p_{i xor k}
SL = np.zeros((4, 4))      # kappa_a = sum_j SL[a^j, j] * G[a^j, j]
SR = np.zeros((4, 4))      # lambda_b = sum_j SR[b^j, j] * H[b^j, j]
for k in range(4):
    for j in range(4):
        a = k ^ j
        G_SGN[k, j] = _Q[j, a, k] * (1 if a == 0 else -1)
        H_SGN[k, j] = _Q[a, k, j] * (1 if a == 0 else -1)
for a in range(4):
    for j in range(4):
        SL[a ^ j, j] = _Q[a, j, a ^ j]
for b in range(4):
    for j in range(4):
        SR[b ^ j, j] = _Q[j, b, b ^ j]

# internal direction order m' -> chi index
MPRIME = [4, 5, 1, 2, 0, 3]
SA = [1.0, 1.0, -1.0, -1.0, 1.0, -1.0]
SB = [1.0, -1.0, 1.0, -1.0, -1.0, -1.0]

# XOR gather: row k of the idx table (k^0, k^1, k^2, k^3) as offset + 2D AP
XOR_AP = {0: (0, 2, 1), 1: (1, 2, -1), 2: (2, -2, 1), 3: (3, -2, -1)}

# ------------- polynomial coefficients: f(h^2) on [0, TMAX] -----------
TMAX = 54.0
NDEG = 8
SEGL = NDEG + 1        # 9 scan entries per segment
NSEG = 3 * B * 2       # (fn, b, t)
SCW = NSEG * SEGL      # 108


def _fit_coeffs():
    from numpy.polynomial import Chebyshev, Polynomial
    t = np.linspace(0, TMAX, 4001)

    def sinc_f(tt):
        h = np.sqrt(np.maximum(tt, 1e-30))
        return np.where(tt < 1e-12, 1.0 - tt / 6, np.sin(h) / np.maximum(h, 1e-30))

    def cos_f(tt):
        return np.cos(np.sqrt(np.maximum(tt, 0)))

    def s2t_f(tt):
        return np.where(tt < 1e-8, -1.0 / 3 + tt / 30,
                        (cos_f(tt) - sinc_f(tt)) / np.maximum(tt, 1e-30))

    out = []
    for f in (sinc_f, cos_f, s2t_f):
        cs = Chebyshev.fit(t, f(t), NDEG)
        p = cs.convert(kind=Polynomial, domain=[0, TMAX], window=[0, TMAX])
        out.append(p.coef)
    return np.asarray(out, F32)      # (3, 9) power basis a0..a8


COEF = _fit_coeffs()

# ---------------- in1 row layout (f32 offsets) ----------------
AB_OFF = 0                 # (b, {al,be}, 3)        12
PP_OFF = AB_OFF + 12       # (b, t)                  4
CB_OFF = PP_OFF + 4        # C blocks (b, 16)       32
UB_OFF = CB_OFF + 32       # U blocks (b, 16)       32
D1_OFF = UB_OFF + 32       # scan coeffs           108
GH_OFF = D1_OFF + SCW      # (s, k, y) sign rows    32
KS_OFF = GH_OFF + 32       # (s, j, a) sign rows    32
SG_OFF = KS_OFF + 32       # (t, m') sign rows      12
PN_OFF = SG_OFF + 12       # (b, t) -pp              4
IN1_W = PN_OFF + 4         # 268


def _const_block() -> np.ndarray:
    c = np.zeros((1, IN1_W), F32)
    # D1: Horner coeffs, segment (fn, b, t), entry s holds a_{NDEG-s}
    for fn in range(3):
        for b in range(B):
            for t in range(2):
                seg = (fn * B + b) * 2 + t
                c[0, D1_OFF + seg * SEGL: D1_OFF + (seg + 1) * SEGL] = COEF[fn, ::-1]
    c[0, GH_OFF:GH_OFF + 16] = G_SGN.reshape(16)
    c[0, GH_OFF + 16:GH_OFF + 32] = H_SGN.reshape(16)
    ks = np.zeros((2, 4, 4), F32)
    for s, Sm in enumerate((SL, SR)):
        for j in range(4):
            for a in range(4):
                ks[s, j, a] = Sm[a ^ j, j]
    c[0, KS_OFF:KS_OFF + 32] = ks.reshape(32)
    c[0, SG_OFF:SG_OFF + 6] = SA
    c[0, SG_OFF + 6:SG_OFF + 12] = SB
    return c


def _ap(base: bass.AP, off: int, *dims) -> bass.AP:
    """Rebuild an AP over `base`'s tensor: partition dim kept, free dims given
    as (stride, size) pairs, offset in elements added to base offset."""
    return bass.AP(tensor=base.tensor, offset=base.offset + off,
                   ap=[base.ap[0]] + [[s, n] for (s, n) in dims])


def tile_body(ctx, tc, outs, ins):
    nc = tc.nc
    A = mybir.AluOpType
    (in1_d,) = ins
    res_d = outs[0]

    pool = ctx.enter_context(tc.tile_pool(name="main", bufs=1))

    def T(tag, *shape):
        return pool.tile([P, *shape], DT, tag=tag, name=tag)

    in1t = T("in1", IN1_W)
    nc.sync.dma_start(in1t[:], in1_d, single_packet=True)
    in1 = in1t[:]

    # ---- w = [a; b] = [al+be; al-be] ----
    w = T("w", B, 2, 3)
    nc.vector.tensor_add(_ap(w[:], 0, (6, B), (1, 3)),
                         _ap(in1,AB_OFF, (6, B), (1, 3)),
                         _ap(in1,AB_OFF + 3, (6, B), (1, 3)))
    nc.vector.tensor_sub(_ap(w[:], 3, (6, B), (1, 3)),
                         _ap(in1,AB_OFF, (6, B), (1, 3)),
                         _ap(in1,AB_OFF + 3, (6, B), (1, 3)))

    # ---- h2 ----
    wsq = T("wsq", B, 2, 3)
    nc.vector.tensor_mul(wsq[:], w[:], w[:])
    h2 = T("h2", B, 2)
    nc.vector.tensor_reduce(out=_ap(h2[:], 0, (2, B), (1, 2), (0, 1)),
                            in_=wsq[:], axis=mybir.AxisListType.X, op=A.add)

    # ---- trig via Horner scan: build data0 = [0, h2 x8] per segment ----
    D0 = T("D0", SCW)
    nc.vector.memset(_ap(D0[:], 0, (SEGL, NSEG)), 0.0)
    nc.vector.tensor_copy(
        _ap(D0[:], 1, (36, 3), (18, B), (9, 2), (1, NDEG)),
        _ap(h2[:], 0, (0, 3), (2, B), (1, 2), (0, NDEG)))
    SCO = T("SCO", SCW)
    nc.vector.tensor_tensor_scan(
        out=SCO[:],
        data0=D0[:],
        data1=_ap(in1, D1_OFF, (1, SCW)),
        initial=0.0, op0=A.mult, op1=A.add)
    # results: snc at 18b+9t+8, cos at 36+18b+9t+8, s2t at 72+18b+9t+8

    # ---- pq2[s, b, e]: s=0 -> q (t=1 half), s=1 -> p (t=0 half) ----
    pq2 = T("pq2", 2, B, 4)
    nc.vector.tensor_copy(_ap(pq2[:], 0, (8, 2), (4, B)),
                          _ap(SCO[:], 36 + 9 + 8, (-9, 2), (18, B)))
    nc.vector.tensor_tensor(_ap(pq2[:], 1, (8, 2), (4, B), (1, 3)),
                            _ap(w[:], 3, (-3, 2), (6, B), (1, 3)),
                            _ap(SCO[:], 9 + 8, (-9, 2), (18, B), (0, 3)),
                            op=A.mult)

    # ---- Z = (W C^T - C^T W) U  via rank-1 structure (on GpSimd, in
    # parallel with the DVE trig chain; only C/U/pp feed this path) ----
    Cpp = T("Cpp", B, 2, 4)      # pp_t * C[k, 2t]
    nc.gpsimd.tensor_tensor(Cpp[:],
                            _ap(in1, CB_OFF, (16, B), (2, 2), (4, 4)),
                            _ap(in1, PP_OFF, (2, B), (1, 2), (0, 4)),
                            op=A.mult)
    vcp = T("vcp", B, 2, 4, 4)   # [b,t,j,k] = Cpp[b,t,k] * U[k,j]
    for t in range(2):
        nc.gpsimd.tensor_tensor(_ap(vcp[:], 16 * t, (32, B), (4, 4), (1, 4)),
                                _ap(Cpp[:], 4 * t, (8, B), (0, 4), (1, 4)),
                                _ap(in1, UB_OFF, (16, B), (1, 4), (4, 4)),
                                op=A.mult)
    vh = T("vh", B, 2, 4, 2)     # pairwise k-sums (gpsimd has no free-axis reduce)
    nc.gpsimd.tensor_add(_ap(vh[:], 0, (16, B), (8, 2), (2, 4), (1, 2)),
                         _ap(vcp[:], 0, (32, B), (16, 2), (4, 4), (1, 2)),
                         _ap(vcp[:], 2, (32, B), (16, 2), (4, 4), (1, 2)))
    vc = T("vc", B, 2, 4)
    nc.gpsimd.tensor_add(_ap(vc[:], 0, (8, B), (4, 2), (1, 4)),
                         _ap(vh[:], 0, (16, B), (8, 2), (2, 4)),
                         _ap(vh[:], 1, (16, B), (8, 2), (2, 4)))
    scb = T("scb", B, 2, 4)      # -pp_t * C[2t+1, i]
    nc.gpsimd.tensor_tensor(scb[:],
                            _ap(in1, CB_OFF + 4, (16, B), (8, 2), (1, 4)),
                            _ap(in1, PN_OFF, (2, B), (1, 2), (0, 4)),
                            op=A.mult)
    Zt0 = T("Zt0", B, 4, 4, 2)   # [b,i,j,t] = scb[b,t,i] * U[2t,j]
    for t in range(2):
        nc.gpsimd.tensor_tensor(_ap(Zt0[:], t, (32, B), (8, 4), (2, 4)),
                                _ap(scb[:], 4 * t, (8, B), (1, 4), (0, 4)),
                                _ap(in1, UB_OFF + 8 * t, (16, B), (0, 4), (1, 4)),
                                op=A.mult)
    Z = T("Z", B, 16)
    nc.gpsimd.tensor_add(_ap(Z[:], 0, (16, B), (4, 4), (1, 4)),
                         _ap(Zt0[:], 0, (32, B), (8, 4), (2, 4)),
                         _ap(Zt0[:], 1, (32, B), (8, 4), (2, 4)))
    # rows 1,3 += vc   (in-place on same engine)
    nc.gpsimd.tensor_add(_ap(Z[:], 4, (16, B), (8, 2), (1, 4)),
                         _ap(Z[:], 4, (16, B), (8, 2), (1, 4)),
                         _ap(vc[:], 0, (8, B), (4, 2), (1, 4)))

    # ---- RL[s,b,k,y]: s=0 R(qbar)[k,y], s=1 L(pbar)[y,k] ----
    RL = T("RL", 2, B, 4, 4)
    for k in range(4):
        off, sA_, sB_ = XOR_AP[k]
        nc.vector.tensor_tensor(
            _ap(RL[:], 4 * k, (16 * B, 2), (16, B), (2, 2), (1, 2)),
            _ap(pq2[:], off, (4 * B, 2), (4, B), (sA_, 2), (sB_, 2)),
            _ap(in1,GH_OFF + 4 * k, (16, 2), (0, B), (2, 2), (1, 2)),
            op=A.mult)

    # ---- G = Z @ R(qbar); H = L(pbar) @ Z  (per-k rank-1 slices) ----
    Gp = T("Gp", B, 4, 4, 4)     # [b,x,y,k] = Z[x,k] * RL[0,b,k,y]
    Hp = T("Hp", B, 4, 4, 4)     # [b,x,y,k] = RL[1,b,k,x] * Z[k,y]
    for k in range(4):
        nc.vector.tensor_tensor(_ap(Gp[:], k, (64, B), (16, 4), (4, 4)),
                                _ap(Z[:], k, (16, B), (4, 4), (0, 4)),
                                _ap(RL[:], 4 * k, (16, B), (0, 4), (1, 4)),
                                op=A.mult)
        nc.vector.tensor_tensor(_ap(Hp[:], k, (64, B), (16, 4), (4, 4)),
                                _ap(RL[:], 16 * B + 4 * k, (16, B), (1, 4), (0, 4)),
                                _ap(Z[:], 4 * k, (16, B), (0, 4), (1, 4)),
                                op=A.mult)
    GH = T("GH", B, 2, 16)       # s=0: G, s=1: H
    nc.vector.tensor_reduce(out=_ap(GH[:], 0, (32, B), (1, 16), (0, 1)),
                            in_=_ap(Gp[:], 0, (64, B), (4, 16), (1, 4)),
                            axis=mybir.AxisListType.X, op=A.add)
    nc.vector.tensor_reduce(out=_ap(GH[:], 16, (32, B), (1, 16), (0, 1)),
                            in_=_ap(Hp[:], 0, (64, B), (4, 16), (1, 4)),
                            axis=mybir.AxisListType.X, op=A.add)

    # ---- kappa/lambda: KLp[s,a,j] = KS[s,j,a] * GH[s, a^j, j] ----
    KLp = T("KLp", B, 2, 4, 4)
    for j in range(4):
        off, sA_, sB_ = XOR_AP[j]
        nc.vector.tensor_tensor(
            _ap(KLp[:], j, (32, B), (16, 2), (8, 2), (4, 2)),
            _ap(GH[:], 4 * off + j, (32, B), (16, 2), (4 * sA_, 2), (4 * sB_, 2)),
            _ap(in1,KS_OFF + 4 * j, (0, B), (16, 2), (2, 2), (1, 2)),
            op=A.mult)
    KL = T("KL", B, 2, 4)        # [s=0]=kappa, [s=1]=lambda
    nc.vector.tensor_reduce(out=_ap(KL[:], 0, (8, B), (1, 8), (0, 1)),
                            in_=_ap(KLp[:], 0, (32, B), (4, 8), (1, 4)),
                            axis=mybir.AxisListType.X, op=A.add)

    # ---- S6: assembly ----
    pr6 = T("pr6", B, 2, 3)
    nc.vector.tensor_tensor(pr6[:], w[:],
                            _ap(KL[:], 1, (8, B), (4, 2), (1, 3)), op=A.mult)
    dot = T("dot", B, 2)
    nc.vector.tensor_reduce(out=_ap(dot[:], 0, (2, B), (1, 2), (0, 1)),
                            in_=pr6[:], axis=mybir.AxisListType.X, op=A.add)
    t6a = T("t6a", B, 2)
    nc.vector.tensor_tensor(t6a[:],
                            _ap(SCO[:], 8, (18, B), (9, 2)),
                            _ap(KL[:], 0, (8, B), (4, 2)), op=A.mult)
    t6b = T("t6b", B, 2)
    nc.vector.tensor_tensor(t6b[:],
                            _ap(SCO[:], 72 + 8, (18, B), (9, 2)),
                            dot[:], op=A.mult)
    Aq = T("Aq", B, 2)
    nc.vector.tensor_sub(Aq[:], t6b[:], t6a[:])
    tm1 = T("tm1", B, 2, 6)
    nc.vector.tensor_tensor(_ap(tm1[:], 0, (12, B), (6, 2), (2, 3), (1, 2)),
                            _ap(Aq[:], 0, (2, B), (1, 2), (0, 3), (0, 2)),
                            _ap(w[:], 0, (6, B), (3, 2), (1, 3), (0, 2)),
                            op=A.mult)
    tm2 = T("tm2", B, 2, 6)
    nc.vector.tensor_tensor(_ap(tm2[:], 0, (12, B), (6, 2), (2, 3), (1, 2)),
                            _ap(SCO[:], 8, (18, B), (9, 2), (0, 3), (0, 2)),
                            _ap(KL[:], 1, (8, B), (4, 2), (1, 3), (0, 2)),
                            op=A.mult)
    tsum = T("tsum", B, 2, 6)
    nc.vector.tensor_add(tsum[:], tm1[:], tm2[:])
    tsgn = T("tsgn", B, 2, 6)
    nc.vector.tensor_tensor(tsgn[:],
                            _ap(tsum[:], 0, (12, B), (6, 2), (1, 6)),
                            _ap(in1,SG_OFF, (0, B), (6, 2), (1, 6)),
                            op=A.mult)
    res = T("res", B, 6)
    nc.vector.tensor_add(_ap(res[:], 0, (6, B), (1, 6)),
                         _ap(tsgn[:], 0, (12, B), (1, 6)),
                         _ap(tsgn[:], 6, (12, B), (1, 6)))
    nc.sync.dma_start(res_d, res[:].rearrange("p a b -> p (a b)"),
                      single_packet=True)


# ---------------- SPMD module build + host wrapper ----------------
_CACHE = {}


def _build_nc():
    nc = bacc.Bacc("TRN2", target_bir_lowering=False)
    in1_d = nc.dram_tensor("in1", [P, IN1_W], DT, kind="ExternalInput")
    res_d = nc.dram_tensor("res", [P, B * 6], DT, kind="ExternalOutput")
    from contextlib import ExitStack
    with tile.TileContext(nc) as tc:
        with ExitStack() as ctx:
            tile_body(ctx, tc, [res_d[:]], [in1_d[:]])
    if not nc.is_finalized():
        nc.finalize()
    return nc


def _prep_in_maps(chi, cov, upd, pcpa):
    g = chi.shape[0]
    k4 = cov.shape[0] // 4
    idx = np.arange(g)
    C = cov.reshape(k4, 4, k4, 4)[idx, :, idx, :].reshape(g, 16).astype(F32)
    U = upd.reshape(k4, 4, k4, 4)[idx, :, idx, :].reshape(g, 16).astype(F32)
    alpha = np.stack([chi[:, 4], -chi[:, 2], -chi[:, 3]], axis=1).astype(F32)
    beta = np.stack([chi[:, 5], -chi[:, 1], chi[:, 0]], axis=1).astype(F32)
    pe = pcpa[0::2].astype(F32)
    po = pcpa[1::2].astype(F32)
    cst = np.broadcast_to(_const_block(), (P, IN1_W))
    in_maps = []
    for core in range(NCORES):
        sl = slice(core * GPC, (core + 1) * GPC)
        in1 = cst.copy()
        abv = in1[:, AB_OFF:AB_OFF + 12].reshape(P, B, 2, 3)
        abv[:, :, 0, :] = alpha[sl].reshape(B, P, 3).transpose(1, 0, 2)
        abv[:, :, 1, :] = beta[sl].reshape(B, P, 3).transpose(1, 0, 2)
        ppv = np.stack([pe[sl].reshape(B, P).T, po[sl].reshape(B, P).T],
                       axis=-1).reshape(P, 4)
        in1[:, PP_OFF:PP_OFF + 4] = ppv
        in1[:, PN_OFF:PN_OFF + 4] = -ppv
        in1[:, CB_OFF:CB_OFF + 32] = C[sl].reshape(B, P, 16).transpose(1, 0, 2).reshape(P, 32)
        in1[:, UB_OFF:UB_OFF + 32] = U[sl].reshape(B, P, 16).transpose(1, 0, 2).reshape(P, 32)
        in_maps.append({"in1": in1})
    return in_maps


def _assemble(results, g):
    out = np.zeros((6, g), F32)
    for core in range(NCORES):
        res = results[core]["res"].reshape(P, B, 6)
        sl = slice(core * GPC, (core + 1) * GPC)
        for t in range(6):
            out[MPRIME[t], sl] = res[:, :, t].T.reshape(GPC)
    return out


def run_spmd(inputs, trace=False, **kw):
    """Run on the 8 neuron cores; returns (out (6,g) f32, BassKernelResults)."""
    if "nc" not in _CACHE:
        _CACHE["nc"] = _build_nc()
    nc = _CACHE["nc"]
    chi = np.asarray(inputs["chi"], F32)
    cov = np.asarray(inputs["covariance_matrix"], F32)
    upd = np.asarray(inputs["update_matrix"], F32)
    pcpa = np.asarray(inputs["partial_cost_partial_activation"], F32)
    in_maps = _prep_in_maps(chi, cov, upd, pcpa)
    br = run_bass_kernel_spmd(nc, in_maps, core_ids=list(range(NCORES)),
                              trace=trace, **kw)
    out = _assemble(br.results, chi.shape[0])
    return out, br


def kernel(**inputs) -> np.ndarray:
    out, _ = run_spmd(inputs, trace=False)
    return out


# revision 20
# speedup vs baseline: 1.1550x; 1.1550x over previous
"""Trainium2 Bass kernel for nn_Brick_Wall (brick-wall gate-layer gradient).

Math: for each gate g the 4x4 antisymmetric E(chi) splits over so(4) =
su(2)+su(2) as E = L(a) + R(b) (left/right quaternion multiplications), so
expm(E) = L(exp_H a) R(exp_H b) in closed form, and the Frechet derivative
contracts against the per-gate matrix Z = (W C^T - C^T W) U down to two
per-gate 4-vectors kappa/lambda.

This version runs the ENTIRE per-gate computation on the Vector engine (DVE)
only: sinc/cos/(cos-sinc)/h^2 are evaluated as degree-8 polynomials in h^2
via a single tensor_tensor_scan (Horner recurrence along the free dim), so
no Scalar-engine activations, no activation-table loads, and no cross-engine
compute synchronization are needed.

Sharding: gates (2048) split contiguously across 8 cores (256 = 2 blocks of
128 partitions each). Host does layout marshaling only (diag-block extraction,
signed column permutations, reshapes); all arithmetic runs on-device.
"""
import sys

for _p in ("/opt/trn_rl_repo",):
    if _p not in sys.path:
        sys.path.insert(0, _p)

import numpy as np

import concourse.bacc as bacc
import concourse.bass as bass
import concourse.tile as tile
from concourse import mybir
from concourse.bass_utils import run_bass_kernel_spmd

F32 = np.float32
P = 128          # partitions (gates per block)
B = 2            # gate blocks per core
NCORES = 8
GPC = P * B      # gates per core
DT = mybir.dt.float32

# ---------------- constant tables (quaternion algebra) ----------------
_Q = np.zeros((4, 4, 4))
for (a, b), (c, s) in {
    (0, 0): (0, 1), (0, 1): (1, 1), (0, 2): (2, 1), (0, 3): (3, 1),
    (1, 0): (1, 1), (1, 1): (0, -1), (1, 2): (3, 1), (1, 3): (2, -1),
    (2, 0): (2, 1), (2, 1): (3, -1), (2, 2): (0, -1), (2, 3): (1, 1),
    (3, 0): (3, 1), (3, 1): (2, 1), (3, 2): (1, -1), (3, 3): (0, -1),
}.items():
    _Q[a, b, c] = s

G_SGN = np.zeros((4, 4))   # R(qbar)[k,j] = G_SGN[k,j] * q_{k xor j}
H_SGN = np.zeros((4, 4))   # L(pbar)[i,k] = H_SGN[k,i] * p_{i xor k}
SL = np.zeros((4, 4))      # kappa_a = sum_j SL[a^j, j] * G[a^j, j]
SR = np.zeros((4, 4))      # lambda_b = sum_j SR[b^j, j] * H[b^j, j]
for k in range(4):
    for j in range(4):
        a = k ^ j
        G_SGN[k, j] = _Q[j, a, k] * (1 if a == 0 else -1)
        H_SGN[k, j] = _Q[a, k, j] * (1 if a == 0 else -1)
for a in range(4):
    for j in range(4):
        SL[a ^ j, j] = _Q[a, j, a ^ j]
for b in range(4):
    for j in range(4):
        SR[b ^ j, j] = _Q[j, b, b ^ j]

# internal direction order m' -> chi index
MPRIME = [4, 5, 1, 2, 0, 3]
SA = [1.0, 1.0, -1.0, -1.0, 1.0, -1.0]
SB = [1.0, -1.0, 1.0, -1.0, -1.0, -1.0]

# XOR gather: row k of the idx table (k^0, k^1, k^2, k^3) as offset + 2D AP
XOR_AP = {0: (0, 2, 1), 1: (1, 2, -1), 2: (2, -2, 1), 3: (3, -2, -1)}

# ------------- polynomial coefficients: f(h^2) on [0, TMAX] -----------
TMAX = 54.0
NDEG = 8
SEGL = NDEG + 1        # 9 scan entries per segment
NSEG = 3 * B * 2       # (fn, b, t)
SCW = NSEG * SEGL      # 108


def _fit_coeffs():
    from numpy.polynomial import Chebyshev, Polynomial
    t = np.linspace(0, TMAX, 4001)

    def sinc_f(tt):
        h = np.sqrt(np.maximum(tt, 1e-30))
        return np.where(tt < 1e-12, 1.0 - tt / 6, np.sin(h) / np.maximum(h, 1e-30))

    def cos_f(tt):
        return np.cos(np.sqrt(np.maximum(tt, 0)))

    def s2t_f(tt):
        return np.where(tt < 1e-8, -1.0 / 3 + tt / 30,
                        (cos_f(tt) - sinc_f(tt)) / np.maximum(tt, 1e-30))

    out = []
    for f in (sinc_f, cos_f, s2t_f):
        cs = Chebyshev.fit(t, f(t), NDEG)
        p = cs.convert(kind=Polynomial, domain=[0, TMAX], window=[0, TMAX])
        out.append(p.coef)
    return np.asarray(out, F32)      # (3, 9) power basis a0..a8


COEF = _fit_coeffs()

# ---------------- in1 row layout (f32 offsets) ----------------
AB_OFF = 0                 # (b, {al,be}, 3)        12
PP_OFF = AB_OFF + 12       # (b, t)                  4
CB_OFF = PP_OFF + 4        # C blocks (b, 16)       32
UB_OFF = CB_OFF + 32       # U blocks (b, 16)       32
D1_OFF = UB_OFF + 32       # scan coeffs           108
GH_OFF = D1_OFF + SCW      # (s, k, y) sign rows    32
KS_OFF = GH_OFF + 32       # (s, j, a) sign rows    32
SG_OFF = KS_OFF + 32       # (t, m') sign rows      12
PN_OFF = SG_OFF + 12       # (b, t) -pp              4
IN1_W = PN_OFF + 4         # 268


def _const_block() -> np.ndarray:
    c = np.zeros((1, IN1_W), F32)
    # D1: Horner coeffs, segment (fn, b, t), entry s holds a_{NDEG-s}
    for fn in range(3):
        for b in range(B):
            for t in range(2):
                seg = (fn * B + b) * 2 + t
                c[0, D1_OFF + seg * SEGL: D1_OFF + (seg + 1) * SEGL] = COEF[fn, ::-1]
    c[0, GH_OFF:GH_OFF + 16] = G_SGN.reshape(16)
    c[0, GH_OFF + 16:GH_OFF + 32] = H_SGN.reshape(16)
    ks = np.zeros((2, 4, 4), F32)
    for s, Sm in enumerate((SL, SR)):
        for j in range(4):
            for a in range(4):
                ks[s, j, a] = Sm[a ^ j, j]
    c[0, KS_OFF:KS_OFF + 32] = ks.reshape(32)
    c[0, SG_OFF:SG_OFF + 6] = SA
    c[0, SG_OFF + 6:SG_OFF + 12] = SB
    return c


def _ap(base: bass.AP, off: int, *dims) -> bass.AP:
    """Rebuild an AP over `base`'s tensor: partition dim kept, free dims given
    as (stride, size) pairs, offset in elements added to base offset."""
    return bass.AP(tensor=base.tensor, offset=base.offset + off,
                   ap=[base.ap[0]] + [[s, n] for (s, n) in dims])


def tile_body(ctx, tc, outs, ins):
    nc = tc.nc
    A = mybir.AluOpType
    (in1_d,) = ins
    res_d = outs[0]

    pool = ctx.enter_context(tc.tile_pool(name="main", bufs=1))

    def T(tag, *shape):
        return pool.tile([P, *shape], DT, tag=tag, name=tag)

    in1t = T("in1", IN1_W)
    nc.sync.dma_start(in1t[:], in1_d, single_packet=True)
    in1 = in1t[:]

    # ---- w = [a; b] = [al+be; al-be] ----
    w = T("w", B, 2, 3)
    nc.vector.tensor_add(_ap(w[:], 0, (6, B), (1, 3)),
                         _ap(in1,AB_OFF, (6, B), (1, 3)),
                         _ap(in1,AB_OFF + 3, (6, B), (1, 3)))
    nc.vector.tensor_sub(_ap(w[:], 3, (6, B), (1, 3)),
                         _ap(in1,AB_OFF, (6, B), (1, 3)),
                         _ap(in1,AB_OFF + 3, (6, B), (1, 3)))

    # ---- h2 ----
    wsq = T("wsq", B, 2, 3)
    nc.vector.tensor_mul(wsq[:], w[:], w[:])
    h2 = T("h2", B, 2)
    nc.vector.tensor_reduce(out=_ap(h2[:], 0, (2, B), (1, 2), (0, 1)),
                            in_=wsq[:], axis=mybir.AxisListType.X, op=A.add)

    # ---- trig via Horner scan: build data0 = [0, h2 x8] per segment ----
    D0 = T("D0", SCW)
    nc.vector.memset(_ap(D0[:], 0, (SEGL, NSEG)), 0.0)
    nc.vector.tensor_copy(
        _ap(D0[:], 1, (36, 3), (18, B), (9, 2), (1, NDEG)),
        _ap(h2[:], 0, (0, 3), (2, B), (1, 2), (0, NDEG)))
    SCO = T("SCO", SCW)
    nc.vector.tensor_tensor_scan(
        out=SCO[:],
        data0=D0[:],
        data1=_ap(in1, D1_OFF, (1, SCW)),
        initial=0.0, op0=A.mult, op1=A.add)
    # results: snc at 18b+9t+8, cos at 36+18b+9t+8, s2t at 72+18b+9t+8

    # ---- pq2[s, b, e]: s=0 -> q (t=1 half), s=1 -> p (t=0 half) ----
    pq2 = T("pq2", 2, B, 4)
    nc.vector.tensor_copy(_ap(pq2[:], 0, (8, 2), (4, B)),
                          _ap(SCO[:], 36 + 9 + 8, (-9, 2), (18, B)))
    nc.vector.tensor_tensor(_ap(pq2[:], 1, (8, 2), (4, B), (1, 3)),
                            _ap(w[:], 3, (-3, 2), (6, B), (1, 3)),
                            _ap(SCO[:], 9 + 8, (-9, 2), (18, B), (0, 3)),
                            op=A.mult)

    # ---- Z = (W C^T - C^T W) U  via rank-1 structure ----
    Cpp = T("Cpp", B, 2, 4)      # pp_t * C[k, 2t]
    nc.vector.tensor_tensor(Cpp[:],
                            _ap(in1, CB_OFF, (16, B), (2, 2), (4, 4)),
                            _ap(in1, PP_OFF, (2, B), (1, 2), (0, 4)),
                            op=A.mult)
    vcp = T("vcp", B, 2, 4, 4)   # [b,t,j,k] = Cpp[b,t,k] * U[k,j]
    for t in range(2):
        nc.vector.tensor_tensor(_ap(vcp[:], 16 * t, (32, B), (4, 4), (1, 4)),
                                _ap(Cpp[:], 4 * t, (8, B), (0, 4), (1, 4)),
                                _ap(in1, UB_OFF, (16, B), (1, 4), (4, 4)),
                                op=A.mult)
    vc = T("vc", B, 2, 4)
    nc.vector.tensor_reduce(out=_ap(vc[:], 0, (8, B), (4, 2), (1, 4), (0, 1)),
                            in_=vcp[:], axis=mybir.AxisListType.X, op=A.add)
    scb = T("scb", B, 2, 4)      # -pp_t * C[2t+1, i]
    nc.vector.tensor_tensor(scb[:],
                            _ap(in1, CB_OFF + 4, (16, B), (8, 2), (1, 4)),
                            _ap(in1, PN_OFF, (2, B), (1, 2), (0, 4)),
                            op=A.mult)
    Zt0 = T("Zt0", B, 4, 4, 2)   # [b,i,j,t] = scb[b,t,i] * U[2t,j]
    for t in range(2):
        nc.vector.tensor_tensor(_ap(Zt0[:], t, (32, B), (8, 4), (2, 4)),
                                _ap(scb[:], 4 * t, (8, B), (1, 4), (0, 4)),
                                _ap(in1, UB_OFF + 8 * t, (16, B), (0, 4), (1, 4)),
                                op=A.mult)
    Z = T("Z", B, 16)
    nc.vector.tensor_add(_ap(Z[:], 0, (16, B), (4, 4), (1, 4)),
                         _ap(Zt0[:], 0, (32, B), (8, 4), (2, 4)),
                         _ap(Zt0[:], 1, (32, B), (8, 4), (2, 4)))
    # rows 1,3 += vc   (in-place on same engine)
    nc.vector.tensor_add(_ap(Z[:], 4, (16, B), (8, 2), (1, 4)),
                         _ap(Z[:], 4, (16, B), (8, 2), (1, 4)),
                         _ap(vc[:], 0, (8, B), (4, 2), (1, 4)))

    # ---- RL[s,b,k,y]: s=0 R(qbar)[k,y], s=1 L(pbar)[y,k] ----
    RL = T("RL", 2, B, 4, 4)
    for k in range(4):
        off, sA_, sB_ = XOR_AP[k]
        nc.vector.tensor_tensor(
            _ap(RL[:], 4 * k, (16 * B, 2), (16, B), (2, 2), (1, 2)),
            _ap(pq2[:], off, (4 * B, 2), (4, B), (sA_, 2), (sB_, 2)),
            _ap(in1,GH_OFF + 4 * k, (16, 2), (0, B), (2, 2), (1, 2)),
            op=A.mult)

    # ---- G = Z @ R(qbar); H = L(pbar) @ Z  (per-k rank-1 slices) ----
    Gp = T("Gp", B, 4, 4, 4)     # [b,x,y,k] = Z[x,k] * RL[0,b,k,y]
    Hp = T("Hp", B, 4, 4, 4)     # [b,x,y,k] = RL[1,b,k,x] * Z[k,y]
    for k in range(4):
        nc.vector.tensor_tensor(_ap(Gp[:], k, (64, B), (16, 4), (4, 4)),
                                _ap(Z[:], k, (16, B), (4, 4), (0, 4)),
                                _ap(RL[:], 4 * k, (16, B), (0, 4), (1, 4)),
                                op=A.mult)
        nc.vector.tensor_tensor(_ap(Hp[:], k, (64, B), (16, 4), (4, 4)),
                                _ap(RL[:], 16 * B + 4 * k, (16, B), (1, 4), (0, 4)),
                                _ap(Z[:], 4 * k, (16, B), (0, 4), (1, 4)),
                                op=A.mult)
    GH = T("GH", B, 2, 16)       # s=0: G, s=1: H
    nc.vector.tensor_reduce(out=_ap(GH[:], 0, (32, B), (1, 16), (0, 1)),
                            in_=_ap(Gp[:], 0, (64, B), (4, 16), (1, 4)),
                            axis=mybir.AxisListType.X, op=A.add)
    nc.vector.tensor_reduce(out=_ap(GH[:], 16, (32, B), (1, 16), (0, 1)),
                            in_=_ap(Hp[:], 0, (64, B), (4, 16), (1, 4)),
                            axis=mybir.AxisListType.X, op=A.add)

    # ---- kappa/lambda: KLp[s,a,j] = KS[s,j,a] * GH[s, a^j, j] ----
    KLp = T("KLp", B, 2, 4, 4)
    for j in range(4):
        off, sA_, sB_ = XOR_AP[j]
        nc.vector.tensor_tensor(
            _ap(KLp[:], j, (32, B), (16, 2), (8, 2), (4, 2)),
            _ap(GH[:], 4 * off + j, (32, B), (16, 2), (4 * sA_, 2), (4 * sB_, 2)),
            _ap(in1,KS_OFF + 4 * j, (0, B), (16, 2), (2, 2), (1, 2)),
            op=A.mult)
    KL = T("KL", B, 2, 4)        # [s=0]=kappa, [s=1]=lambda
    nc.vector.tensor_reduce(out=_ap(KL[:], 0, (8, B), (1, 8), (0, 1)),
                            in_=_ap(KLp[:], 0, (32, B), (4, 8), (1, 4)),
                            axis=mybir.AxisListType.X, op=A.add)

    # ---- S6: assembly ----
    pr6 = T("pr6", B, 2, 3)
    nc.vector.tensor_tensor(pr6[:], w[:],
                            _ap(KL[:], 1, (8, B), (4, 2), (1, 3)), op=A.mult)
    dot = T("dot", B, 2)
    nc.vector.tensor_reduce(out=_ap(dot[:], 0, (2, B), (1, 2), (0, 1)),
                            in_=pr6[:], axis=mybir.AxisListType.X, op=A.add)
    t6a = T("t6a", B, 2)
    nc.vector.tensor_tensor(t6a[:],
                            _ap(SCO[:], 8, (18, B), (9, 2)),
                            _ap(KL[:], 0, (8, B), (4, 2)), op=A.mult)
    t6b = T("t6b", B, 2)
    nc.vector.tensor_tensor(t6b[:],
                            _ap(SCO[:], 72 + 8, (18, B), (9, 2)),
                            dot[:], op=A.mult)
    Aq = T("Aq", B, 2)
    nc.vector.tensor_sub(Aq[:], t6b[:], t6a[:])
    tm1 = T("tm1", B, 2, 6)
    nc.vector.tensor_tensor(_ap(tm1[:], 0, (12, B), (6, 2), (2, 3), (1, 2)),
                            _ap(Aq[:], 0, (2, B), (1, 2), (0, 3), (0, 2)),
                            _ap(w[:], 0, (6, B), (3, 2), (1, 3), (0, 2)),
                            op=A.mult)
    tm2 = T("tm2", B, 2, 6)
    nc.vector.tensor_tensor(_ap(tm2[:], 0, (12, B), (6, 2), (2, 3), (1, 2)),
                            _ap(SCO[:], 8, (18, B), (9, 2), (0, 3), (0, 2)),
                            _ap(KL[:], 1, (8, B), (4, 2), (1, 3), (0, 2)),
                            op=A.mult)
    tsum = T("tsum", B, 2, 6)
    nc.vector.tensor_add(tsum[:], tm1[:], tm2[:])
    tsgn = T("tsgn", B, 2, 6)
    nc.vector.tensor_tensor(tsgn[:],
                            _ap(tsum[:], 0, (12, B), (6, 2), (1, 6)),
                            _ap(in1,SG_OFF, (0, B), (6, 2), (1, 6)),
                            op=A.mult)
    res = T("res", B, 6)
    nc.vector.tensor_add(_ap(res[:], 0, (6, B), (1, 6)),
                         _ap(tsgn[:], 0, (12, B), (1, 6)),
                         _ap(tsgn[:], 6, (12, B), (1, 6)))
    nc.sync.dma_start(res_d, res[:].rearrange("p a b -> p (a b)"),
                      single_packet=True)


# ---------------- SPMD module build + host wrapper ----------------
_CACHE = {}


def _build_nc():
    nc = bacc.Bacc("TRN2", target_bir_lowering=False)
    in1_d = nc.dram_tensor("in1", [P, IN1_W], DT, kind="ExternalInput")
    res_d = nc.dram_tensor("res", [P, B * 6], DT, kind="ExternalOutput")
    from contextlib import ExitStack
    with tile.TileContext(nc) as tc:
        with ExitStack() as ctx:
            tile_body(ctx, tc, [res_d[:]], [in1_d[:]])
    if not nc.is_finalized():
        nc.finalize()
    return nc


def _prep_in_maps(chi, cov, upd, pcpa):
    g = chi.shape[0]
    k4 = cov.shape[0] // 4
    idx = np.arange(g)
    C = cov.reshape(k4, 4, k4, 4)[idx, :, idx, :].reshape(g, 16).astype(F32)
    U = upd.reshape(k4, 4, k4, 4)[idx, :, idx, :].reshape(g, 16).astype(F32)
    alpha = np.stack([chi[:, 4], -chi[:, 2], -chi[:, 3]], axis=1).astype(F32)
    beta = np.stack([chi[:, 5], -chi[:, 1], chi[:, 0]], axis=1).astype(F32)
    pe = pcpa[0::2].astype(F32)
    po = pcpa[1::2].astype(F32)
    cst = np.broadcast_to(_const_block(), (P, IN1_W))
    in_maps = []
    for core in range(NCORES):
        sl = slice(core * GPC, (core + 1) * GPC)
        in1 = cst.copy()
        abv = in1[:, AB_OFF:AB_OFF + 12].reshape(P, B, 2, 3)
        abv[:, :, 0, :] = alpha[sl].reshape(B, P, 3).transpose(1, 0, 2)
        abv[:, :, 1, :] = beta[sl].reshape(B, P, 3).transpose(1, 0, 2)
        ppv = np.stack([pe[sl].reshape(B, P).T, po[sl].reshape(B, P).T],
                       axis=-1).reshape(P, 4)
        in1[:, PP_OFF:PP_OFF + 4] = ppv
        in1[:, PN_OFF:PN_OFF + 4] = -ppv
        in1[:, CB_OFF:CB_OFF + 32] = C[sl].reshape(B, P, 16).transpose(1, 0, 2).reshape(P, 32)
        in1[:, UB_OFF:UB_OFF + 32] = U[sl].reshape(B, P, 16).transpose(1, 0, 2).reshape(P, 32)
        in_maps.append({"in1": in1})
    return in_maps


def _assemble(results, g):
    out = np.zeros((6, g), F32)
    for core in range(NCORES):
        res = results[core]["res"].reshape(P, B, 6)
        sl = slice(core * GPC, (core + 1) * GPC)
        for t in range(6):
            out[MPRIME[t], sl] = res[:, :, t].T.reshape(GPC)
    return out


def run_spmd(inputs, trace=False, **kw):
    """Run on the 8 neuron cores; returns (out (6,g) f32, BassKernelResults)."""
    if "nc" not in _CACHE:
        _CACHE["nc"] = _build_nc()
    nc = _CACHE["nc"]
    chi = np.asarray(inputs["chi"], F32)
    cov = np.asarray(inputs["covariance_matrix"], F32)
    upd = np.asarray(inputs["update_matrix"], F32)
    pcpa = np.asarray(inputs["partial_cost_partial_activation"], F32)
    in_maps = _prep_in_maps(chi, cov, upd, pcpa)
    br = run_bass_kernel_spmd(nc, in_maps, core_ids=list(range(NCORES)),
                              trace=trace, **kw)
    out = _assemble(br.results, chi.shape[0])
    return out, br


def kernel(**inputs) -> np.ndarray:
    out, _ = run_spmd(inputs, trace=False)
    return out


# revision 31
# speedup vs baseline: 1.4322x; 1.2399x over previous
"""Trainium2 Bass kernel for nn_Brick_Wall (brick-wall gate-layer gradient).

Math: for each gate g the 4x4 antisymmetric E(chi) splits over so(4) =
su(2)+su(2) as E = L(a) + R(b) (left/right quaternion multiplications), so
expm(E) = L(exp_H a) R(exp_H b) in closed form, and the Frechet derivative
contracts against the per-gate matrix Z = (W C^T - C^T W) U down to two
per-gate 4-vectors kappa/lambda.

This version runs the ENTIRE per-gate computation on the Vector engine (DVE)
only: sinc/cos/(cos-sinc)/h^2 are evaluated as degree-8 polynomials in h^2
via a single tensor_tensor_scan (Horner recurrence along the free dim), so
no Scalar-engine activations, no activation-table loads, and no cross-engine
compute synchronization are needed.

Sharding: gates (2048) split contiguously across 8 cores (256 = 2 blocks of
128 partitions each). Host does layout marshaling only (diag-block extraction,
signed column permutations, reshapes); all arithmetic runs on-device.
"""
import sys

for _p in ("/opt/trn_rl_repo",):
    if _p not in sys.path:
        sys.path.insert(0, _p)

import numpy as np

import concourse.bacc as bacc
import concourse.bass as bass
import concourse.tile as tile
from concourse import mybir
from concourse.bass_utils import run_bass_kernel_spmd

F32 = np.float32
P = 128          # partitions (gates per block)
B = 2            # gate blocks per core
NCORES = 8
GPC = P * B      # gates per core
DT = mybir.dt.float32

# ---------------- constant tables (quaternion algebra) ----------------
_Q = np.zeros((4, 4, 4))
for (a, b), (c, s) in {
    (0, 0): (0, 1), (0, 1): (1, 1), (0, 2): (2, 1), (0, 3): (3, 1),
    (1, 0): (1, 1), (1, 1): (0, -1), (1, 2): (3, 1), (1, 3): (2, -1),
    (2, 0): (2, 1), (2, 1): (3, -1), (2, 2): (0, -1), (2, 3): (1, 1),
    (3, 0): (3, 1), (3, 1): (2, 1), (3, 2): (1, -1), (3, 3): (0, -1),
}.items():
    _Q[a, b, c] = s

G_SGN = np.zeros((4, 4))   # R(qbar)[k,j] = G_SGN[k,j] * q_{k xor j}
H_SGN = np.zeros((4, 4))   # L(pbar)[i,k] = H_SGN[k,i] * p_{i xor k}
SL = np.zeros((4, 4))      # kappa_a = sum_j SL[a^j, j] * G[a^j, j]
SR = np.zeros((4, 4))      # lambda_b = sum_j SR[b^j, j] * H[b^j, j]
for k in range(4):
    for j in range(4):
        a = k ^ j
        G_SGN[k, j] = _Q[j, a, k] * (1 if a == 0 else -1)
        H_SGN[k, j] = _Q[a, k, j] * (1 if a == 0 else -1)
for a in range(4):
    for j in range(4):
        SL[a ^ j, j] = _Q[a, j, a ^ j]
for b in range(4):
    for j in range(4):
        SR[b ^ j, j] = _Q[j, b, b ^ j]

# internal direction order m' -> chi index
MPRIME = [4, 5, 1, 2, 0, 3]
SA = [1.0, 1.0, -1.0, -1.0, 1.0, -1.0]
SB = [1.0, -1.0, 1.0, -1.0, -1.0, -1.0]

# XOR gather: row k of the idx table (k^0, k^1, k^2, k^3) as offset + 2D AP
XOR_AP = {0: (0, 2, 1), 1: (1, 2, -1), 2: (2, -2, 1), 3: (3, -2, -1)}

# ------------- polynomial coefficients: f(h^2) on [0, TMAX] -----------
TMAX = 54.0
NDEG = 8
SEGL = NDEG + 1        # 9 scan entries per segment
NSEG = 3 * B * 2       # (fn, b, t)
SCW = NSEG * SEGL      # 108


def _fit_coeffs():
    from numpy.polynomial import Chebyshev, Polynomial
    t = np.linspace(0, TMAX, 4001)

    def sinc_f(tt):
        h = np.sqrt(np.maximum(tt, 1e-30))
        return np.where(tt < 1e-12, 1.0 - tt / 6, np.sin(h) / np.maximum(h, 1e-30))

    def cos_f(tt):
        return np.cos(np.sqrt(np.maximum(tt, 0)))

    def s2t_f(tt):
        return np.where(tt < 1e-8, -1.0 / 3 + tt / 30,
                        (cos_f(tt) - sinc_f(tt)) / np.maximum(tt, 1e-30))

    out = []
    for f in (sinc_f, cos_f, s2t_f):
        cs = Chebyshev.fit(t, f(t), NDEG)
        p = cs.convert(kind=Polynomial, domain=[0, TMAX], window=[0, TMAX])
        out.append(p.coef)
    return np.asarray(out, F32)      # (3, 9) power basis a0..a8


COEF = _fit_coeffs()

# ---------------- in1 row layout (f32 offsets) ----------------
AL_OFF = 0                 # alpha (b, 3)            6
BN_OFF = AL_OFF + 6        # [beta; -beta] (b,t,3)  12
PP_OFF = BN_OFF + 12       # (b, t)                  4
CB_OFF = PP_OFF + 4        # C blocks (b, 16)       32
UB_OFF = CB_OFF + 32       # U blocks (b, 16)       32
D1_OFF = UB_OFF + 32       # scan coeffs           108
D0_OFF = D1_OFF + SCW      # scan data0 zeros      108
GH_OFF = D0_OFF + SCW      # (s, k, y) sign rows    32
KS_OFF = GH_OFF + 32       # (s, j, a) sign rows    32
SG_OFF = KS_OFF + 32       # (t, m') sign rows      12
PN_OFF = SG_OFF + 12       # (b, t) -pp              4
IN1_W = PN_OFF + 4         # 376


def _const_block() -> np.ndarray:
    c = np.zeros((1, IN1_W), F32)
    # D1: Horner coeffs, segment (fn, b, t), entry s holds a_{NDEG-s}
    for fn in range(3):
        for b in range(B):
            for t in range(2):
                seg = (fn * B + b) * 2 + t
                c[0, D1_OFF + seg * SEGL: D1_OFF + (seg + 1) * SEGL] = COEF[fn, ::-1]
    c[0, GH_OFF:GH_OFF + 16] = G_SGN.reshape(16)
    c[0, GH_OFF + 16:GH_OFF + 32] = H_SGN.reshape(16)
    ks = np.zeros((2, 4, 4), F32)
    for s, Sm in enumerate((SL, SR)):
        for j in range(4):
            for a in range(4):
                ks[s, j, a] = Sm[a ^ j, j]
    c[0, KS_OFF:KS_OFF + 32] = ks.reshape(32)
    c[0, SG_OFF:SG_OFF + 6] = SA
    c[0, SG_OFF + 6:SG_OFF + 12] = SB
    return c


def _ap(base: bass.AP, off: int, *dims) -> bass.AP:
    """Rebuild an AP over `base`'s tensor: partition dim kept, free dims given
    as (stride, size) pairs, offset in elements added to base offset."""
    return bass.AP(tensor=base.tensor, offset=base.offset + off,
                   ap=[base.ap[0]] + [[s, n] for (s, n) in dims])


def tile_body(ctx, tc, outs, ins):
    nc = tc.nc
    A = mybir.AluOpType
    (in1_d,) = ins
    res_d = outs[0]

    pool = ctx.enter_context(tc.tile_pool(name="main", bufs=1))

    def T(tag, *shape):
        return pool.tile([P, *shape], DT, tag=tag, name=tag)

    in1t = T("in1", IN1_W)
    nc.sync.dma_start(in1t[:], in1_d, single_packet=True)
    in1 = in1t[:]

    # ---- w = [al+be; al-be] in one op (payload ships [be; -be]) ----
    w = T("w", B, 2, 3)
    nc.vector.tensor_add(w[:],
                         _ap(in1, AL_OFF, (3, B), (0, 2), (1, 3)),
                         _ap(in1, BN_OFF, (6, B), (3, 2), (1, 3)))

    # ---- h2 ----
    wsq = T("wsq", B, 2, 3)
    nc.vector.tensor_mul(wsq[:], w[:], w[:])
    h2 = T("h2", B, 2)
    nc.vector.tensor_reduce(out=_ap(h2[:], 0, (2, B), (1, 2), (0, 1)),
                            in_=wsq[:], axis=mybir.AxisListType.X, op=A.add)

    # ---- trig via Horner scan: data0 = [0, h2 x8] per segment; the
    # zero slots arrive with the DMA payload, h2 is copied in here ----
    nc.vector.tensor_copy(
        _ap(in1, D0_OFF + 1, (36, 3), (18, B), (9, 2), (1, NDEG)),
        _ap(h2[:], 0, (0, 3), (2, B), (1, 2), (0, NDEG)))
    SCO = T("SCO", SCW)
    nc.vector.tensor_tensor_scan(
        out=SCO[:],
        data0=_ap(in1, D0_OFF, (1, SCW)),
        data1=_ap(in1, D1_OFF, (1, SCW)),
        initial=0.0, op0=A.mult, op1=A.add)
    # results: snc at 18b+9t+8, cos at 36+18b+9t+8, s2t at 72+18b+9t+8

    # ---- Z = (W C^T - C^T W) U  via rank-1 structure ----
    Cpp = T("Cpp", B, 2, 4)      # pp_t * C[k, 2t]
    nc.vector.tensor_tensor(Cpp[:],
                            _ap(in1, CB_OFF, (16, B), (2, 2), (4, 4)),
                            _ap(in1, PP_OFF, (2, B), (1, 2), (0, 4)),
                            op=A.mult)
    vcp = T("vcp", B, 2, 4, 4)   # [b,t,j,k] = Cpp[b,t,k] * U[k,j]
    for t in range(2):
        nc.vector.tensor_tensor(_ap(vcp[:], 16 * t, (32, B), (4, 4), (1, 4)),
                                _ap(Cpp[:], 4 * t, (8, B), (0, 4), (1, 4)),
                                _ap(in1, UB_OFF, (16, B), (1, 4), (4, 4)),
                                op=A.mult)
    vc = T("vc", B, 2, 4)
    nc.vector.tensor_reduce(out=_ap(vc[:], 0, (8, B), (4, 2), (1, 4), (0, 1)),
                            in_=vcp[:], axis=mybir.AxisListType.X, op=A.add)
    scb = T("scb", B, 2, 4)      # -pp_t * C[2t+1, i]
    nc.vector.tensor_tensor(scb[:],
                            _ap(in1, CB_OFF + 4, (16, B), (8, 2), (1, 4)),
                            _ap(in1, PN_OFF, (2, B), (1, 2), (0, 4)),
                            op=A.mult)
    Zt0 = T("Zt0", B, 4, 4, 2)   # [b,i,j,t] = scb[b,t,i] * U[2t,j]
    for t in range(2):
        nc.vector.tensor_tensor(_ap(Zt0[:], t, (32, B), (8, 4), (2, 4)),
                                _ap(scb[:], 4 * t, (8, B), (1, 4), (0, 4)),
                                _ap(in1, UB_OFF + 8 * t, (16, B), (0, 4), (1, 4)),
                                op=A.mult)
    Z = T("Z", B, 16)
    nc.vector.tensor_add(_ap(Z[:], 0, (16, B), (4, 4), (1, 4)),
                         _ap(Zt0[:], 0, (32, B), (8, 4), (2, 4)),
                         _ap(Zt0[:], 1, (32, B), (8, 4), (2, 4)))
    # rows 1,3 += vc   (in-place on same engine)
    nc.vector.tensor_add(_ap(Z[:], 4, (16, B), (8, 2), (1, 4)),
                         _ap(Z[:], 4, (16, B), (8, 2), (1, 4)),
                         _ap(vc[:], 0, (8, B), (4, 2), (1, 4)))

    # ---- pq2[s, b, e]: s=0 -> q (t=1 half), s=1 -> p (t=0 half) ----
    pq2 = T("pq2", 2, B, 4)
    nc.vector.tensor_copy(_ap(pq2[:], 0, (8, 2), (4, B)),
                          _ap(SCO[:], 36 + 9 + 8, (-9, 2), (18, B)))
    nc.vector.tensor_tensor(_ap(pq2[:], 1, (8, 2), (4, B), (1, 3)),
                            _ap(w[:], 3, (-3, 2), (6, B), (1, 3)),
                            _ap(SCO[:], 9 + 8, (-9, 2), (18, B), (0, 3)),
                            op=A.mult)

    # ---- RL[s,b,k,y]: s=0 R(qbar)[k,y], s=1 L(pbar)[y,k] ----
    RL = T("RL", 2, B, 4, 4)
    for k in range(4):
        off, sA_, sB_ = XOR_AP[k]
        nc.vector.tensor_tensor(
            _ap(RL[:], 4 * k, (16 * B, 2), (16, B), (2, 2), (1, 2)),
            _ap(pq2[:], off, (4 * B, 2), (4, B), (sA_, 2), (sB_, 2)),
            _ap(in1,GH_OFF + 4 * k, (16, 2), (0, B), (2, 2), (1, 2)),
            op=A.mult)

    # ---- G = Z @ R(qbar); H = L(pbar) @ Z  (per-k rank-1 slices) ----
    Gp = T("Gp", B, 4, 4, 4)     # [b,x,y,k] = Z[x,k] * RL[0,b,k,y]
    Hp = T("Hp", B, 4, 4, 4)     # [b,x,y,k] = RL[1,b,k,x] * Z[k,y]
    for k in range(4):
        nc.vector.tensor_tensor(_ap(Gp[:], k, (64, B), (16, 4), (4, 4)),
                                _ap(Z[:], k, (16, B), (4, 4), (0, 4)),
                                _ap(RL[:], 4 * k, (16, B), (0, 4), (1, 4)),
                                op=A.mult)
        nc.vector.tensor_tensor(_ap(Hp[:], k, (64, B), (16, 4), (4, 4)),
                                _ap(RL[:], 16 * B + 4 * k, (16, B), (1, 4), (0, 4)),
                                _ap(Z[:], 4 * k, (16, B), (0, 4), (1, 4)),
                                op=A.mult)
    GH = T("GH", B, 2, 16)       # s=0: G, s=1: H
    nc.vector.tensor_reduce(out=_ap(GH[:], 0, (32, B), (1, 16), (0, 1)),
                            in_=_ap(Gp[:], 0, (64, B), (4, 16), (1, 4)),
                            axis=mybir.AxisListType.X, op=A.add)
    nc.vector.tensor_reduce(out=_ap(GH[:], 16, (32, B), (1, 16), (0, 1)),
                            in_=_ap(Hp[:], 0, (64, B), (4, 16), (1, 4)),
                            axis=mybir.AxisListType.X, op=A.add)

    # ---- kappa/lambda: KLp[s,a,j] = KS[s,j,a] * GH[s, a^j, j] ----
    KLp = T("KLp", B, 2, 4, 4)
    for j in range(4):
        off, sA_, sB_ = XOR_AP[j]
        nc.vector.tensor_tensor(
            _ap(KLp[:], j, (32, B), (16, 2), (8, 2), (4, 2)),
            _ap(GH[:], 4 * off + j, (32, B), (16, 2), (4 * sA_, 2), (4 * sB_, 2)),
            _ap(in1,KS_OFF + 4 * j, (0, B), (16, 2), (2, 2), (1, 2)),
            op=A.mult)
    KL = T("KL", B, 2, 4)        # [s=0]=kappa, [s=1]=lambda
    nc.vector.tensor_reduce(out=_ap(KL[:], 0, (8, B), (1, 8), (0, 1)),
                            in_=_ap(KLp[:], 0, (32, B), (4, 8), (1, 4)),
                            axis=mybir.AxisListType.X, op=A.add)

    # ---- S6: assembly ----
    pr6 = T("pr6", B, 2, 3)
    nc.vector.tensor_tensor(pr6[:], w[:],
                            _ap(KL[:], 1, (8, B), (4, 2), (1, 3)), op=A.mult)
    t6a = T("t6a", B, 2)
    nc.vector.tensor_tensor(t6a[:],
                            _ap(SCO[:], 8, (18, B), (9, 2)),
                            _ap(KL[:], 0, (8, B), (4, 2)), op=A.mult)
    dot = T("dot", B, 2)
    nc.vector.tensor_reduce(out=_ap(dot[:], 0, (2, B), (1, 2), (0, 1)),
                            in_=pr6[:], axis=mybir.AxisListType.X, op=A.add)
    tm2 = T("tm2", B, 2, 6)
    nc.vector.tensor_tensor(_ap(tm2[:], 0, (12, B), (6, 2), (2, 3), (1, 2)),
                            _ap(SCO[:], 8, (18, B), (9, 2), (0, 3), (0, 2)),
                            _ap(KL[:], 1, (8, B), (4, 2), (1, 3), (0, 2)),
                            op=A.mult)
    t6b = T("t6b", B, 2)
    nc.vector.tensor_tensor(t6b[:],
                            _ap(SCO[:], 72 + 8, (18, B), (9, 2)),
                            dot[:], op=A.mult)
    Aq = T("Aq", B, 2)
    nc.vector.tensor_sub(Aq[:], t6b[:], t6a[:])
    tm1 = T("tm1", B, 2, 6)
    nc.vector.tensor_tensor(_ap(tm1[:], 0, (12, B), (6, 2), (2, 3), (1, 2)),
                            _ap(Aq[:], 0, (2, B), (1, 2), (0, 3), (0, 2)),
                            _ap(w[:], 0, (6, B), (3, 2), (1, 3), (0, 2)),
                            op=A.mult)
    tsum = T("tsum", B, 2, 6)
    nc.vector.tensor_add(tsum[:], tm1[:], tm2[:])
    tsgn = T("tsgn", B, 2, 6)
    nc.vector.tensor_tensor(tsgn[:],
                            _ap(tsum[:], 0, (12, B), (6, 2), (1, 6)),
                            _ap(in1,SG_OFF, (0, B), (6, 2), (1, 6)),
                            op=A.mult)
    res = T("res", B, 6)
    nc.vector.tensor_add(_ap(res[:], 0, (6, B), (1, 6)),
                         _ap(tsgn[:], 0, (12, B), (1, 6)),
                         _ap(tsgn[:], 6, (12, B), (1, 6)))
    nc.sync.dma_start(res_d, res[:].rearrange("p a b -> p (a b)"))


# ---------------- SPMD module build + host wrapper ----------------
_CACHE = {}


def _prune_dead_preamble(nc):
    """Drop the framework's const-tile memsets (this kernel never reads the
    const APs — no activations, no const_ap users) and the all-engine
    barrier that only exists to order engines after those memsets. The
    barrier instance is self-contained (gather +4 / release +4), so removing
    the whole set keeps the barrier semaphores balanced. This lets the input
    DMA issue right after the sync engine's preamble instead of waiting for
    the Pool engine's memsets."""
    mb = nc.m.functions[0].blocks[0]
    def dead(ins):
        tn = type(ins).__name__
        s = str(ins)
        if tn == "InstMemset" and "const-" in s:
            return True
        if tn in ("InstDrain", "InstEventSemaphore") and "barrier_" in s:
            return True
        return False
    mb.instructions[:] = [i for i in mb.instructions if not dead(i)]


def _build_nc():
    nc = bacc.Bacc("TRN2", target_bir_lowering=False)
    _prune_dead_preamble(nc)
    in1_d = nc.dram_tensor("in1", [P, IN1_W], DT, kind="ExternalInput")
    res_d = nc.dram_tensor("res", [P, B * 6], DT, kind="ExternalOutput")
    from contextlib import ExitStack
    with tile.TileContext(nc) as tc:
        with ExitStack() as ctx:
            tile_body(ctx, tc, [res_d[:]], [in1_d[:]])
    if not nc.is_finalized():
        nc.finalize()
    return nc


def _prep_in_maps(chi, cov, upd, pcpa):
    g = chi.shape[0]
    k4 = cov.shape[0] // 4
    idx = np.arange(g)
    C = cov.reshape(k4, 4, k4, 4)[idx, :, idx, :].reshape(g, 16).astype(F32)
    U = upd.reshape(k4, 4, k4, 4)[idx, :, idx, :].reshape(g, 16).astype(F32)
    alpha = np.stack([chi[:, 4], -chi[:, 2], -chi[:, 3]], axis=1).astype(F32)
    beta = np.stack([chi[:, 5], -chi[:, 1], chi[:, 0]], axis=1).astype(F32)
    pe = pcpa[0::2].astype(F32)
    po = pcpa[1::2].astype(F32)
    cst = np.broadcast_to(_const_block(), (P, IN1_W))
    in_maps = []
    for core in range(NCORES):
        sl = slice(core * GPC, (core + 1) * GPC)
        in1 = cst.copy()
        alv = alpha[sl].reshape(B, P, 3).transpose(1, 0, 2)
        bev = beta[sl].reshape(B, P, 3).transpose(1, 0, 2)
        in1[:, AL_OFF:AL_OFF + 6] = alv.reshape(P, 6)
        bn = in1[:, BN_OFF:BN_OFF + 12].reshape(P, B, 2, 3)
        bn[:, :, 0, :] = bev
        bn[:, :, 1, :] = -bev
        ppv = np.stack([pe[sl].reshape(B, P).T, po[sl].reshape(B, P).T],
                       axis=-1).reshape(P, 4)
        in1[:, PP_OFF:PP_OFF + 4] = ppv
        in1[:, PN_OFF:PN_OFF + 4] = -ppv
        in1[:, CB_OFF:CB_OFF + 32] = C[sl].reshape(B, P, 16).transpose(1, 0, 2).reshape(P, 32)
        in1[:, UB_OFF:UB_OFF + 32] = U[sl].reshape(B, P, 16).transpose(1, 0, 2).reshape(P, 32)
        in_maps.append({"in1": in1})
    return in_maps


def _assemble(results, g):
    out = np.zeros((6, g), F32)
    for core in range(NCORES):
        res = results[core]["res"].reshape(P, B, 6)
        sl = slice(core * GPC, (core + 1) * GPC)
        for t in range(6):
            out[MPRIME[t], sl] = res[:, :, t].T.reshape(GPC)
    return out


def run_spmd(inputs, trace=False, **kw):
    """Run on the 8 neuron cores; returns (out (6,g) f32, BassKernelResults)."""
    if "nc" not in _CACHE:
        _CACHE["nc"] = _build_nc()
    nc = _CACHE["nc"]
    chi = np.asarray(inputs["chi"], F32)
    cov = np.asarray(inputs["covariance_matrix"], F32)
    upd = np.asarray(inputs["update_matrix"], F32)
    pcpa = np.asarray(inputs["partial_cost_partial_activation"], F32)
    in_maps = _prep_in_maps(chi, cov, upd, pcpa)
    br = run_bass_kernel_spmd(nc, in_maps, core_ids=list(range(NCORES)),
                              trace=trace, **kw)
    out = _assemble(br.results, chi.shape[0])
    return out, br


def kernel(**inputs) -> np.ndarray:
    out, _ = run_spmd(inputs, trace=False)
    return out
